# revision 2
# baseline (speedup 1.0000x reference)
"""GCN ConvBlock (GCNConv + LayerNorm) on 8 Trainium2 NeuronCores.

Math: out = LayerNorm(A_hat @ x @ W + b) * gamma + beta, with
A_hat = D^-1/2 (A + I) D^-1/2 over N=10000 nodes / E=640000 edges.

v2 changes vs v1:
  - W is folded into the stationary operand on the host:
    xw = (dinv[:,None] * x) @ W, so the kernel computes
    aggT[f, d] = sum_s xw[s, f] * C[s, d], then out = LN(dinv_d * aggT^T).
    This removes the per-iteration W matmuls; the [f,d]->[d,f] transpose
    is done with an identity-rhs matmul (128 moving cols per dst tile).
  - mode="dr": the A matmul runs in fp8 DoubleRow perf mode (both operands
    fp8e4m3, two 128-deep k-tiles per pass, 2 moving cols/cycle).  xw is
    split hi+lo into two fp8 operands accumulated into the same PSUM to
    recover ~bf16 accuracy (host-simulated rel err ~4.6e-3).
  - mode="mixed": bf16 stationary xw, fp8 moving C (v1's A matmul).
"""

import numpy as np
import ml_dtypes

N = 10000
E = 640000
D = 128
EPS = 1e-5

NCORES = 8
DST_PER_CORE = 1250
DST_PAD = 1280

BF16 = ml_dtypes.bfloat16
FP8 = ml_dtypes.float8_e4m3

MODE = "dr"               # "dr" | "mixed"
# lo-correction coverage: the lo fp8 pass runs over the first LO_PAIRS
# pair-blocks of the importance-sorted row order (40 = full coverage).
# Host-sim rel err: 40 -> 0.0049, 24 -> 0.0148, 20 -> 0.0169 (gate 2e-2).
LO_PAIRS = 20

# mixed mode: 79 src blocks; dr mode: 80 blocks = 40 pairs
SRC_BLOCKS = 79
SRC_BLOCKS_DR = 80
SRC_PAD = SRC_BLOCKS * 128        # 10112
SRC_PAD_DR = SRC_BLOCKS_DR * 128  # 10240

CHUNKS_MIX = [(0, 512), (512, 512), (1024, 226)]
# DR: PSUM tiles are bank-granular (2KB = 512 f32); matmul out <= 256 cols in
# DoubleRow, so two 256-col accumulation chains share each 512-col bank tile.
DR_TSZ = [512, 512, 226]
DR_CH = [(0, 0, 0, 256), (0, 256, 256, 256), (1, 0, 512, 256),
         (1, 256, 768, 256), (2, 0, 1024, 226)]   # (tile, tile_off, col_off, n)
DR_FIRST = {0, 2, 4}
DR_LAST = {1, 3, 4}
T_ROWS = [128] * 9 + [98]

_nc_cache = {}


def build_nc(n_iter=1, mode=MODE, lo_pairs=LO_PAIRS, enable_asserts=False):
    key = (n_iter, mode, lo_pairs, enable_asserts)
    if key in _nc_cache:
        return _nc_cache[key]
    import concourse.tile as tile
    from concourse import bacc, mybir

    f32 = mybir.dt.float32
    bf16 = mybir.dt.bfloat16
    fp8 = mybir.dt.float8e4

    nc = bacc.Bacc("TRN2", target_bir_lowering=False, debug=False,
                   enable_asserts=enable_asserts, num_devices=NCORES)

    nsb = SRC_BLOCKS_DR if mode == "dr" else SRC_BLOCKS
    ab_d = nc.dram_tensor("ab", [128, nsb * DST_PAD], fp8, kind="ExternalInput").ap()
    if mode == "dr":
        xh_d = nc.dram_tensor("xh", [128, nsb * 128], fp8, kind="ExternalInput").ap()
        xl_d = nc.dram_tensor("xl", [128, nsb * 128], fp8, kind="ExternalInput").ap()
    else:
        xb_d = nc.dram_tensor("xb", [128, nsb * 128], bf16, kind="ExternalInput").ap()
    id_d = nc.dram_tensor("idm", [128, 128], bf16, kind="ExternalInput").ap()
    dv_d = nc.dram_tensor("dv", [128, DST_PER_CORE], f32, kind="ExternalInput").ap()
    bb_d = nc.dram_tensor("bb", [128, 128], f32, kind="ExternalInput").ap()
    gb_d = nc.dram_tensor("gb", [128, 128], f32, kind="ExternalInput").ap()
    be_d = nc.dram_tensor("be", [128, 128], f32, kind="ExternalInput").ap()
    out_d = nc.dram_tensor("out", [DST_PAD, 128], f32, kind="ExternalOutput").ap()

    with tile.TileContext(nc) as tc:
        with (
            tc.tile_pool(name="const", bufs=1) as cpool,
            tc.tile_pool(name="work", bufs=2) as wpool,
            tc.tile_pool(name="ln", bufs=4) as lpool,
            tc.tile_pool(name="psA", bufs=2, space="PSUM") as psA,
            tc.tile_pool(name="psO", bufs=2, space="PSUM") as psO,
        ):
            if mode == "dr":
                xh = cpool.tile([128, nsb * 128], fp8)
                nc.sync.dma_start(xh, xh_d)
                xl = cpool.tile([128, nsb * 128], fp8)
                nc.scalar.dma_start(xl, xl_d)
            else:
                xb = cpool.tile([128, nsb * 128], bf16)
                nc.sync.dma_start(xb, xb_d)
            idm = cpool.tile([128, 128], bf16)
            nc.scalar.dma_start(idm, id_d)
            dv = cpool.tile([128, DST_PER_CORE], f32)
            nc.scalar.dma_start(dv, dv_d)
            bb = cpool.tile([128, 128], f32)
            nc.scalar.dma_start(bb, bb_d)
            gb = cpool.tile([128, 128], f32)
            nc.scalar.dma_start(gb, gb_d)
            be = cpool.tile([128, 128], f32)
            nc.scalar.dma_start(be, be_d)
            eps_t = cpool.tile([128, 1], f32)
            nc.vector.memset(eps_t, EPS)
            cfull = cpool.tile([128, nsb * DST_PAD], fp8)

            npacks = 10
            pack = [nsb // npacks] * npacks
            for i in range(nsb - sum(pack)):
                pack[i] += 1

            for _it in range(n_iter):
                if mode == "dr":
                    ps = [psA.tile([128, sz], f32, tag=f"ps{ti}", name=f"ps{ti}")
                          for ti, sz in enumerate(DR_TSZ)]
                    dr = mybir.MatmulPerfMode.DoubleRow
                    npair = nsb // 2
                    if _it == 0:
                        nc.sync.dma_start(cfull, ab_d)
                    cv = cfull.rearrange("p (blk d) -> p blk d", blk=nsb)
                    for half, xsrc, hp in ((0, xh, npair), (1, xl, lo_pairs)):
                        xv = xsrc.rearrange("p (blk f) -> p blk f", blk=nsb)
                        for pb in range(hp):
                            lhs = xv[:, 2 * pb:2 * pb + 2, :]
                            for ci, (ti, toff, off, sz) in enumerate(DR_CH):
                                rhs = cv[:, 2 * pb:2 * pb + 2, off:off + sz]
                                nc.tensor.matmul(
                                    ps[ti][:, toff:toff + sz], lhsT=lhs, rhs=rhs,
                                    start=(half == 0 and pb == 0 and ci in DR_FIRST),
                                    stop=(half == 1 and pb == hp - 1
                                          and ci in DR_LAST),
                                    perf_mode=dr)
                else:
                    ps = [psA.tile([128, sz], f32, tag=f"ps{ci}", name=f"ps{ci}")
                          for ci, (_off, sz) in enumerate(CHUNKS_MIX)]
                    sb0 = 0
                    for pk, npk in enumerate(pack):
                        if _it == 0:
                            nc.sync.dma_start(
                                cfull[:, sb0 * DST_PAD:(sb0 + npk) * DST_PAD],
                                ab_d[:, sb0 * DST_PAD:(sb0 + npk) * DST_PAD])
                        for j in range(npk):
                            sb = sb0 + j
                            lhs = xb[:, sb * 128:(sb + 1) * 128]
                            for ci, (off, sz) in enumerate(CHUNKS_MIX):
                                nc.tensor.matmul(
                                    ps[ci][:], lhsT=lhs,
                                    rhs=cfull[:, sb * DST_PAD + off:
                                              sb * DST_PAD + off + sz],
                                    start=(sb == 0), stop=(sb == nsb - 1))
                        sb0 += npk

                # za[f, d] = aggT * dinv[d]  (PSUM -> SBUF bf16)
                za = wpool.tile([128, DST_PER_CORE], bf16, tag="za", name="za")
                for ci, (off, sz) in enumerate(CHUNKS_MIX):
                    nc.vector.tensor_mul(za[:, off:off + sz], ps[ci][:],
                                         dv[:, off:off + sz])
                for t in range(10):
                    rows = T_ROWS[t]
                    cw = min(128, DST_PER_CORE - t * 128)
                    po = psO.tile([128, 128], f32, tag="po", name="po")
                    # out[d, f] = za[:, tile].T  via identity moving operand
                    nc.tensor.matmul(po[:rows, :],
                                     lhsT=za[:, t * 128:t * 128 + cw],
                                     rhs=idm, start=True, stop=True)
                    zb = lpool.tile([128, 128], f32, tag="zb", name="zb")
                    nc.vector.tensor_add(zb[:rows], po[:rows, :], bb[:rows])
                    st = lpool.tile([128, 6], f32, tag="st", name="st")
                    nc.vector.bn_stats(st[:rows], zb[:rows])
                    mv = lpool.tile([128, 2], f32, tag="mv", name="mv")
                    nc.vector.bn_aggr(mv[:rows], st[:rows])
                    rs = lpool.tile([128, 1], f32, tag="rs", name="rs")
                    nc.scalar.activation(
                        out=rs[:rows], in_=mv[:rows, 1:2],
                        func=mybir.ActivationFunctionType.Sqrt,
                        bias=eps_t[:rows], scale=1.0,
                    )
                    nc.vector.reciprocal(rs[:rows], rs[:rows])
                    zn = lpool.tile([128, 128], f32, tag="zn", name="zn")
                    nc.vector.tensor_scalar(
                        out=zn[:rows], in0=zb[:rows], scalar1=mv[:rows, 0:1],
                        scalar2=rs[:rows],
                        op0=mybir.AluOpType.subtract,
                        op1=mybir.AluOpType.mult,
                    )
                    nc.vector.tensor_mul(zn[:rows], zn[:rows], gb[:rows])
                    nc.vector.tensor_add(zn[:rows], zn[:rows], be[:rows])
                    nc.scalar.dma_start(out_d[t * 128:t * 128 + rows, :], zn[:rows])

    nc.compile()
    _nc_cache[key] = nc
    return nc


def _build_count_matrix(src, dst, src_pad):
    C = np.zeros((src_pad, N), np.float32)
    try:
        import scipy.sparse as sp
        ones = np.ones(src.shape[0], np.float32)
        M = sp.coo_matrix((ones, (src, dst)), shape=(src_pad, N)).tocsr()
        C[:] = M.toarray()
    except Exception:
        np.add.at(C, (src, dst), 1.0)
    C[np.arange(N), np.arange(N)] += 1.0
    return C


def _pack_rows(a, nsb):
    """[nsb*128, w] -> [128, nsb*w] block-major packing."""
    w = a.shape[1]
    return np.ascontiguousarray(
        a.reshape(nsb, 128, w).transpose(1, 0, 2).reshape(128, nsb * w))


def prepare_in_maps(x, edge_index, W, b, gamma, beta, mode=MODE):
    """Shard/route per-core inputs.  In dr mode, src rows are permuted by
    descending lo-correction importance (outdeg * ||fp8 residual||^2) so a
    truncated lo pass (LO_PAIRS < 40) corrects the rows that matter most."""
    x = np.asarray(x, np.float32)
    W = np.asarray(W, np.float32)
    b = np.asarray(b, np.float32)
    gamma = np.asarray(gamma, np.float32)
    beta = np.asarray(beta, np.float32)
    src = np.asarray(edge_index[0], np.int64)
    dst = np.asarray(edge_index[1], np.int64)

    deg = np.bincount(dst, minlength=N).astype(np.float32) + 1.0
    dinv = (1.0 / np.sqrt(deg)).astype(np.float32)

    nsb = SRC_BLOCKS_DR if mode == "dr" else SRC_BLOCKS
    src_pad = nsb * 128

    xw = (x * dinv[:, None]) @ W

    common = {}
    if mode == "dr":
        x_hi8 = xw.astype(FP8)
        x_lo = xw - x_hi8.astype(np.float32)
        outdeg = np.bincount(src, minlength=N).astype(np.float32) + 1.0
        imp = outdeg * (x_lo.astype(np.float32) ** 2).sum(1)
        order = np.argsort(-imp).astype(np.int64)
        inv_perm = np.empty(N, np.int64)
        inv_perm[order] = np.arange(N)
        src = inv_perm[src]          # relabel src nodes into sorted order
        self_src = inv_perm          # self loop row for dst d is inv_perm[d]
        C = np.zeros((src_pad, N), np.float32)
        try:
            import scipy.sparse as sp
            M = sp.coo_matrix((np.ones(len(src), np.float32), (src, dst)),
                              shape=(src_pad, N)).tocsr()
            C[:] = M.toarray()
        except Exception:
            np.add.at(C, (src, dst), 1.0)
        C[self_src, np.arange(N)] += 1.0
        xw_full = np.zeros((src_pad, D), np.float32)
        xw_full[:N] = xw[order]
        x_hi = xw_full.astype(FP8)
        x_lo_full = (xw_full - x_hi.astype(np.float32)).astype(FP8)
        common["xh"] = _pack_rows(x_hi.astype(np.float32), nsb).astype(FP8)
        common["xl"] = _pack_rows(x_lo_full.astype(np.float32), nsb).astype(FP8)
    else:
        C = _build_count_matrix(src, dst, src_pad)
        xw_full = np.zeros((src_pad, D), np.float32)
        xw_full[:N] = xw
        common["xb"] = _pack_rows(xw_full, nsb).astype(BF16)
    common["idm"] = np.eye(128, dtype=np.float32).astype(BF16)
    common["bb"] = np.ascontiguousarray(np.broadcast_to(b, (128, 128))).astype(np.float32)
    common["gb"] = np.ascontiguousarray(np.broadcast_to(gamma, (128, 128))).astype(np.float32)
    common["be"] = np.ascontiguousarray(np.broadcast_to(beta, (128, 128))).astype(np.float32)

    in_maps = []
    for c in range(NCORES):
        Ac = np.zeros((src_pad, DST_PAD), np.float32)
        Ac[:, :DST_PER_CORE] = C[:, c * DST_PER_CORE:(c + 1) * DST_PER_CORE]
        ab = _pack_rows(Ac, nsb).astype(FP8)
        dvv = dinv[c * DST_PER_CORE:(c + 1) * DST_PER_CORE]
        dvb = np.ascontiguousarray(np.broadcast_to(dvv, (128, DST_PER_CORE)))
        m = {"ab": ab, "dv": dvb}
        m.update(common)
        in_maps.append(m)
    return in_maps


def assemble_output(results):
    parts = []
    for c in range(NCORES):
        o = np.asarray(results[c]["out"], np.float32)
        parts.append(o[:DST_PER_CORE])
    return np.ascontiguousarray(np.concatenate(parts, axis=0))


def kernel(x, edge_index, W, b, gamma, beta):
    from concourse.bass_utils import run_bass_kernel_spmd

    nc = build_nc()
    in_maps = prepare_in_maps(x, edge_index, W, b, gamma, beta)
    res = run_bass_kernel_spmd(nc, in_maps, core_ids=list(range(NCORES)))
    return assemble_output(res.results)


# revision 4
# speedup vs baseline: 1.3540x; 1.3540x over previous
"""GCN ConvBlock (GCNConv + LayerNorm) on 8 Trainium2 NeuronCores.

Math: out = LayerNorm(A_hat @ x @ W + b) * gamma + beta, with
A_hat = D^-1/2 (A + I) D^-1/2 over N=10000 nodes / E=640000 edges.

v2 changes vs v1:
  - W is folded into the stationary operand on the host:
    xw = (dinv[:,None] * x) @ W, so the kernel computes
    aggT[f, d] = sum_s xw[s, f] * C[s, d], then out = LN(dinv_d * aggT^T).
    This removes the per-iteration W matmuls; the [f,d]->[d,f] transpose
    is done with an identity-rhs matmul (128 moving cols per dst tile).
  - mode="dr": the A matmul runs in fp8 DoubleRow perf mode (both operands
    fp8e4m3, two 128-deep k-tiles per pass, 2 moving cols/cycle).  xw is
    split hi+lo into two fp8 operands accumulated into the same PSUM to
    recover ~bf16 accuracy (host-simulated rel err ~4.6e-3).
  - mode="mixed": bf16 stationary xw, fp8 moving C (v1's A matmul).
"""

import numpy as np
import ml_dtypes

N = 10000
E = 640000
D = 128
EPS = 1e-5

NCORES = 8
DST_PER_CORE = 1250
DST_PAD = 1280

BF16 = ml_dtypes.bfloat16
FP8 = ml_dtypes.float8_e4m3

MODE = "dr"               # "dr" | "mixed"
# lo-correction coverage: the lo fp8 pass runs over the first LO_PAIRS
# pair-blocks of the importance-sorted row order (40 = full coverage).
# Host-sim rel err: 40 -> 0.0049, 24 -> 0.0148, 20 -> 0.0169 (gate 2e-2).
LO_PAIRS = 20

# mixed mode: 79 src blocks; dr mode: 80 blocks = 40 pairs
SRC_BLOCKS = 79
SRC_BLOCKS_DR = 80
SRC_PAD = SRC_BLOCKS * 128        # 10112
SRC_PAD_DR = SRC_BLOCKS_DR * 128  # 10240

CHUNKS_MIX = [(0, 512), (512, 512), (1024, 226)]
# DR: PSUM tiles are bank-granular (2KB = 512 f32); matmul out <= 256 cols in
# DoubleRow, so two 256-col accumulation chains share each 512-col bank tile.
DR_TSZ = [512, 512, 226]
DR_CH = [(0, 0, 0, 256), (0, 256, 256, 256), (1, 0, 512, 256),
         (1, 256, 768, 256), (2, 0, 1024, 226)]   # (tile, tile_off, col_off, n)
DR_FIRST = {0, 2, 4}
DR_LAST = {1, 3, 4}
T_ROWS = [128] * 9 + [98]

_nc_cache = {}


def build_nc(n_iter=1, mode=MODE, lo_pairs=LO_PAIRS, out_q="spread",
             enable_asserts=False):
    key = (n_iter, mode, lo_pairs, out_q, enable_asserts)
    if key in _nc_cache:
        return _nc_cache[key]
    import concourse.tile as tile
    from concourse import bacc, mybir

    f32 = mybir.dt.float32
    bf16 = mybir.dt.bfloat16
    fp8 = mybir.dt.float8e4

    nc = bacc.Bacc("TRN2", target_bir_lowering=False, debug=False,
                   enable_asserts=enable_asserts, num_devices=NCORES)

    nsb = SRC_BLOCKS_DR if mode == "dr" else SRC_BLOCKS
    ab_d = nc.dram_tensor("ab", [128, nsb * DST_PAD], fp8, kind="ExternalInput").ap()
    if mode == "dr":
        xh_d = nc.dram_tensor("xh", [128, nsb * 128], fp8, kind="ExternalInput").ap()
        xl_d = nc.dram_tensor("xl", [128, nsb * 128], fp8, kind="ExternalInput").ap()
    else:
        xb_d = nc.dram_tensor("xb", [128, nsb * 128], bf16, kind="ExternalInput").ap()
    id_d = nc.dram_tensor("idm", [128, 128], bf16, kind="ExternalInput").ap()
    dv_d = nc.dram_tensor("dv", [128, DST_PER_CORE], f32, kind="ExternalInput").ap()
    bb_d = nc.dram_tensor("bb", [128, 128], f32, kind="ExternalInput").ap()
    gb_d = nc.dram_tensor("gb", [128, 128], f32, kind="ExternalInput").ap()
    be_d = nc.dram_tensor("be", [128, 128], f32, kind="ExternalInput").ap()
    out_d = nc.dram_tensor("out", [DST_PAD, 128], f32, kind="ExternalOutput").ap()

    with tile.TileContext(nc) as tc:
        with (
            tc.tile_pool(name="const", bufs=1) as cpool,
            tc.tile_pool(name="work", bufs=2) as wpool,
            tc.tile_pool(name="ln", bufs=4) as lpool,
            tc.tile_pool(name="psA", bufs=2, space="PSUM") as psA,
            tc.tile_pool(name="psO", bufs=2, space="PSUM") as psO,
        ):
            if mode == "dr":
                xh = cpool.tile([128, nsb * 128], fp8)
                nc.sync.dma_start(xh, xh_d)
                xl = cpool.tile([128, nsb * 128], fp8)
                nc.scalar.dma_start(xl, xl_d)
            else:
                xb = cpool.tile([128, nsb * 128], bf16)
                nc.sync.dma_start(xb, xb_d)
            idm = cpool.tile([128, 128], bf16)
            nc.scalar.dma_start(idm, id_d)
            dv = cpool.tile([128, DST_PER_CORE], f32)
            nc.scalar.dma_start(dv, dv_d)
            bb = cpool.tile([128, 128], f32)
            nc.scalar.dma_start(bb, bb_d)
            gb = cpool.tile([128, 128], f32)
            nc.scalar.dma_start(gb, gb_d)
            be = cpool.tile([128, 128], f32)
            nc.scalar.dma_start(be, be_d)
            eps_t = cpool.tile([128, 1], f32)
            nc.vector.memset(eps_t, EPS)
            cfull = cpool.tile([128, nsb * DST_PAD], fp8)

            npacks = 10
            pack = [nsb // npacks] * npacks
            for i in range(nsb - sum(pack)):
                pack[i] += 1

            for _it in range(n_iter):
                if mode == "dr":
                    ps = [psA.tile([128, sz], f32, tag=f"ps{ti}", name=f"ps{ti}")
                          for ti, sz in enumerate(DR_TSZ)]
                    dr = mybir.MatmulPerfMode.DoubleRow
                    npair = nsb // 2
                    if _it == 0:
                        nc.sync.dma_start(cfull, ab_d)
                    cv = cfull.rearrange("p (blk d) -> p blk d", blk=nsb)
                    for half, xsrc, hp in ((0, xh, npair), (1, xl, lo_pairs)):
                        xv = xsrc.rearrange("p (blk f) -> p blk f", blk=nsb)
                        for pb in range(hp):
                            lhs = xv[:, 2 * pb:2 * pb + 2, :]
                            for ci, (ti, toff, off, sz) in enumerate(DR_CH):
                                rhs = cv[:, 2 * pb:2 * pb + 2, off:off + sz]
                                nc.tensor.matmul(
                                    ps[ti][:, toff:toff + sz], lhsT=lhs, rhs=rhs,
                                    start=(half == 0 and pb == 0 and ci in DR_FIRST),
                                    stop=(half == 1 and pb == hp - 1
                                          and ci in DR_LAST),
                                    perf_mode=dr)
                else:
                    ps = [psA.tile([128, sz], f32, tag=f"ps{ci}", name=f"ps{ci}")
                          for ci, (_off, sz) in enumerate(CHUNKS_MIX)]
                    sb0 = 0
                    for pk, npk in enumerate(pack):
                        if _it == 0:
                            nc.sync.dma_start(
                                cfull[:, sb0 * DST_PAD:(sb0 + npk) * DST_PAD],
                                ab_d[:, sb0 * DST_PAD:(sb0 + npk) * DST_PAD])
                        for j in range(npk):
                            sb = sb0 + j
                            lhs = xb[:, sb * 128:(sb + 1) * 128]
                            for ci, (off, sz) in enumerate(CHUNKS_MIX):
                                nc.tensor.matmul(
                                    ps[ci][:], lhsT=lhs,
                                    rhs=cfull[:, sb * DST_PAD + off:
                                              sb * DST_PAD + off + sz],
                                    start=(sb == 0), stop=(sb == nsb - 1))
                        sb0 += npk

                # za[f, d] = aggT * dinv[d]  (PSUM -> SBUF bf16)
                za = wpool.tile([128, DST_PER_CORE], bf16, tag="za", name="za")
                for ci, (off, sz) in enumerate(CHUNKS_MIX):
                    nc.vector.tensor_mul(za[:, off:off + sz], ps[ci][:],
                                         dv[:, off:off + sz])
                for t in range(10):
                    rows = T_ROWS[t]
                    cw = min(128, DST_PER_CORE - t * 128)
                    po = psO.tile([128, 128], f32, tag="po", name="po")
                    # out[d, f] = za[:, tile].T  via identity moving operand
                    nc.tensor.matmul(po[:rows, :],
                                     lhsT=za[:, t * 128:t * 128 + cw],
                                     rhs=idm, start=True, stop=True)
                    zb = lpool.tile([128, 128], f32, tag="zb", name="zb")
                    nc.vector.tensor_add(zb[:rows], po[:rows, :], bb[:rows])
                    st = lpool.tile([128, 6], f32, tag="st", name="st")
                    nc.vector.bn_stats(st[:rows], zb[:rows])
                    mv = lpool.tile([128, 2], f32, tag="mv", name="mv")
                    nc.vector.bn_aggr(mv[:rows], st[:rows])
                    rs = lpool.tile([128, 1], f32, tag="rs", name="rs")
                    nc.scalar.activation(
                        out=rs[:rows], in_=mv[:rows, 1:2],
                        func=mybir.ActivationFunctionType.Sqrt,
                        bias=eps_t[:rows], scale=1.0,
                    )
                    nc.vector.reciprocal(rs[:rows], rs[:rows])
                    zn = lpool.tile([128, 128], f32, tag="zn", name="zn")
                    nc.vector.tensor_scalar(
                        out=zn[:rows], in0=zb[:rows], scalar1=mv[:rows, 0:1],
                        scalar2=rs[:rows],
                        op0=mybir.AluOpType.subtract,
                        op1=mybir.AluOpType.mult,
                    )
                    nc.vector.tensor_mul(zn[:rows], zn[:rows], gb[:rows])
                    nc.vector.tensor_add(zn[:rows], zn[:rows], be[:rows])
                    if out_q == "none" and not (_it == n_iter - 1):
                        continue  # diagnostic only: skip per-iter stores
                    eng = {0: nc.scalar, 1: nc.sync, 2: nc.gpsimd}[
                        t % 3 if out_q == "spread" else 0]
                    eng.dma_start(out_d[t * 128:t * 128 + rows, :], zn[:rows])

    nc.compile()
    _nc_cache[key] = nc
    return nc


def _build_count_matrix(src, dst, src_pad):
    C = np.zeros((src_pad, N), np.float32)
    try:
        import scipy.sparse as sp
        ones = np.ones(src.shape[0], np.float32)
        M = sp.coo_matrix((ones, (src, dst)), shape=(src_pad, N)).tocsr()
        C[:] = M.toarray()
    except Exception:
        np.add.at(C, (src, dst), 1.0)
    C[np.arange(N), np.arange(N)] += 1.0
    return C


def _pack_rows(a, nsb):
    """[nsb*128, w] -> [128, nsb*w] block-major packing."""
    w = a.shape[1]
    return np.ascontiguousarray(
        a.reshape(nsb, 128, w).transpose(1, 0, 2).reshape(128, nsb * w))


def prepare_in_maps(x, edge_index, W, b, gamma, beta, mode=MODE):
    """Shard/route per-core inputs.  In dr mode, src rows are permuted by
    descending lo-correction importance (outdeg * ||fp8 residual||^2) so a
    truncated lo pass (LO_PAIRS < 40) corrects the rows that matter most."""
    x = np.asarray(x, np.float32)
    W = np.asarray(W, np.float32)
    b = np.asarray(b, np.float32)
    gamma = np.asarray(gamma, np.float32)
    beta = np.asarray(beta, np.float32)
    src = np.asarray(edge_index[0], np.int64)
    dst = np.asarray(edge_index[1], np.int64)

    deg = np.bincount(dst, minlength=N).astype(np.float32) + 1.0
    dinv = (1.0 / np.sqrt(deg)).astype(np.float32)

    nsb = SRC_BLOCKS_DR if mode == "dr" else SRC_BLOCKS
    src_pad = nsb * 128

    xw = (x * dinv[:, None]) @ W

    common = {}
    if mode == "dr":
        x_hi8 = xw.astype(FP8)
        x_lo = xw - x_hi8.astype(np.float32)
        outdeg = np.bincount(src, minlength=N).astype(np.float32) + 1.0
        imp = outdeg * (x_lo.astype(np.float32) ** 2).sum(1)
        order = np.argsort(-imp).astype(np.int64)
        inv_perm = np.empty(N, np.int64)
        inv_perm[order] = np.arange(N)
        src = inv_perm[src]          # relabel src nodes into sorted order
        self_src = inv_perm          # self loop row for dst d is inv_perm[d]
        C = np.zeros((src_pad, N), np.float32)
        try:
            import scipy.sparse as sp
            M = sp.coo_matrix((np.ones(len(src), np.float32), (src, dst)),
                              shape=(src_pad, N)).tocsr()
            C[:] = M.toarray()
        except Exception:
            np.add.at(C, (src, dst), 1.0)
        C[self_src, np.arange(N)] += 1.0
        xw_full = np.zeros((src_pad, D), np.float32)
        xw_full[:N] = xw[order]
        x_hi = xw_full.astype(FP8)
        x_lo_full = (xw_full - x_hi.astype(np.float32)).astype(FP8)
        common["xh"] = _pack_rows(x_hi.astype(np.float32), nsb).astype(FP8)
        common["xl"] = _pack_rows(x_lo_full.astype(np.float32), nsb).astype(FP8)
    else:
        C = _build_count_matrix(src, dst, src_pad)
        xw_full = np.zeros((src_pad, D), np.float32)
        xw_full[:N] = xw
        common["xb"] = _pack_rows(xw_full, nsb).astype(BF16)
    common["idm"] = np.eye(128, dtype=np.float32).astype(BF16)
    common["bb"] = np.ascontiguousarray(np.broadcast_to(b, (128, 128))).astype(np.float32)
    common["gb"] = np.ascontiguousarray(np.broadcast_to(gamma, (128, 128))).astype(np.float32)
    common["be"] = np.ascontiguousarray(np.broadcast_to(beta, (128, 128))).astype(np.float32)

    in_maps = []
    for c in range(NCORES):
        Ac = np.zeros((src_pad, DST_PAD), np.float32)
        Ac[:, :DST_PER_CORE] = C[:, c * DST_PER_CORE:(c + 1) * DST_PER_CORE]
        ab = _pack_rows(Ac, nsb).astype(FP8)
        dvv = dinv[c * DST_PER_CORE:(c + 1) * DST_PER_CORE]
        dvb = np.ascontiguousarray(np.broadcast_to(dvv, (128, DST_PER_CORE)))
        m = {"ab": ab, "dv": dvb}
        m.update(common)
        in_maps.append(m)
    return in_maps


def assemble_output(results):
    parts = []
    for c in range(NCORES):
        o = np.asarray(results[c]["out"], np.float32)
        parts.append(o[:DST_PER_CORE])
    return np.ascontiguousarray(np.concatenate(parts, axis=0))


def kernel(x, edge_index, W, b, gamma, beta):
    from concourse.bass_utils import run_bass_kernel_spmd

    nc = build_nc()
    in_maps = prepare_in_maps(x, edge_index, W, b, gamma, beta)
    res = run_bass_kernel_spmd(nc, in_maps, core_ids=list(range(NCORES)))
    return assemble_output(res.results)
